# revision 18
# baseline (speedup 1.0000x reference)
"""Trainium2 Bass kernel for AttnGatingExpL2 (additive attention + cross gating + exp-neg-L2).

Math (per batch b):
  qa[a,q]   = sum_d Wq[d,a] * audio[q,d]          (PE, audio^T staged on host)
  kb[a,k]   = sum_d Wkv[d,a] * text[k,d] + b[a]
  score[k,q]= sum_a v[a] * tanh(qa[a,q] + kb[a,k])  (ACT tanh w/ per-partition bias; PE matvec)
  E'        = select(W, exp(score), A)   -- reproduces reference masking exactly:
              valid (q<alen,k<tlen): exp(score); invalid k: 0; masked q: 1 for ALL k
              (=> attn uniform 1/32 over all 32 keys, as NEG-masked softmax gives)
  denom[q]  = sum_k E'[k,q]  (PE ones-matmul, broadcast to 128 partitions)
  snipT     = text^T @ E'    (unnormalized snippet, transposed world [d,q])
  g_uT      = sigmoid(fuw^T q-proj), g_sT = sigmoid((fsw^T snipT) * (1/denom))
  u=audioT*g_sT, s=snipT*g_uT (unnorm; denom cancels in cosine)
  out[q]    = exp(-sqrt(2-2*us/sqrt(uu*ss)))   (ln/exp form, division-free)

Sharding: data-parallel, 2 batches per core on 8 cores; batches sorted by audio_len
into slot0 (large, q-budget=ceil(max/128) tiles) and slot1 (small) so the tanh stage
only covers ceil(alen)x tlen budgets (trace-time constants; program is SPMD-identical).
"""

import numpy as np
from contextlib import ExitStack

import concourse.bass as bass
import concourse.tile as tile
from concourse import bacc, mybir
from concourse import bass_utils

F32 = mybir.dt.float32
F32R = mybir.dt.float32r
BF16 = mybir.dt.bfloat16
AF = mybir.ActivationFunctionType

NC = 8
B, LQ, LKV, D = 16, 512, 32, 512
NEGBIG = -1.0e10


def _mm(nc, out, lhsT, rhs, start, stop):
    nc.tensor.matmul(out, lhsT, rhs, start=start, stop=stop)


def build_program(QT, TK):
    """QT: [q-tiles slot0, slot1]; TK: [k-budget slot0, slot1]. Returns (nc, names)."""
    nc = bacc.Bacc("TRN2", target_bir_lowering=False, debug=False, num_devices=NC)

    # per-core DRAM inputs (values differ per core, names shared — SPMD)
    d_audioT = nc.dram_tensor("audioT", [2, D, LQ], F32, kind="ExternalInput").ap()
    d_text = nc.dram_tensor("text", [2, LKV, D], F32, kind="ExternalInput").ap()
    d_textT = nc.dram_tensor("textT", [2, D, LKV], F32, kind="ExternalInput").ap()
    d_wm = nc.dram_tensor("wmask", [2, LKV, LQ], mybir.dt.int32, kind="ExternalInput").ap()
    d_am = nc.dram_tensor("amask", [2, LKV, LQ], F32, kind="ExternalInput").ap()
    d_h2w = nc.dram_tensor("h2w", [D, 2 * D], F32, kind="ExternalInput").ap()
    d_fuwT = nc.dram_tensor("fuwT", [D, D], F32, kind="ExternalInput").ap()
    d_fswT = nc.dram_tensor("fswT", [D, D], F32, kind="ExternalInput").ap()
    d_vke = nc.dram_tensor("vke", [128, 4, LKV, 2, LKV], BF16, kind="ExternalInput").ap()
    d_bm = nc.dram_tensor("bm", [128, 4], F32, kind="ExternalInput").ap()
    d_fubh = nc.dram_tensor("fubh", [128, 4], F32, kind="ExternalInput").ap()
    d_fsbh = nc.dram_tensor("fsbh", [128, 4], F32, kind="ExternalInput").ap()
    d_out = nc.dram_tensor("out", [2, LQ], F32, kind="ExternalOutput").ap()

    es = ExitStack()
    with tile.TileContext(nc) as tc:
        with es:
            cpool = es.enter_context(tc.tile_pool(name="consts", bufs=1))
            apool = es.enter_context(tc.tile_pool(name="acts", bufs=2))
            hpool = es.enter_context(tc.tile_pool(name="h", bufs=2))
            spool = es.enter_context(tc.tile_pool(name="small", bufs=1))
            ppool = es.enter_context(tc.tile_pool(name="ps", bufs=2, space=bass.MemorySpace.PSUM))
            scps = es.enter_context(tc.tile_pool(name="scps", bufs=2, space=bass.MemorySpace.PSUM))
            redps = es.enter_context(tc.tile_pool(name="redps", bufs=1, space=bass.MemorySpace.PSUM))

            # ---- constants (once per core) ----
            wq = cpool.tile([128, 4, D], F32)   # [d_in_tile, dt, a]
            wkv = cpool.tile([128, 4, D], F32)
            fuwT = cpool.tile([128, 4, D], F32)  # [d, dt, e]
            fswT = cpool.tile([128, 4, D], F32)
            for dt in range(4):
                nc.sync.dma_start(wq[:, dt, :], d_h2w[bass.ts(dt, 128), 0:D])
                nc.sync.dma_start(wkv[:, dt, :], d_h2w[bass.ts(dt, 128), D:2 * D])
                nc.sync.dma_start(fuwT[:, dt, :], d_fuwT[bass.ts(dt, 128), :])
                nc.sync.dma_start(fswT[:, dt, :], d_fswT[bass.ts(dt, 128), :])
            vke = cpool.tile([128, 4, LKV, 2, LKV], BF16)
            nc.sync.dma_start(vke[:], d_vke[:])
            zrow = cpool.tile([1, 512], BF16)
            nc.vector.memset(zrow[:], 0.0)
            zw = cpool.tile([1, LKV], BF16)
            nc.vector.memset(zw[:], 0.0)
            bm = cpool.tile([128, 4], F32)
            fubh = cpool.tile([128, 4], F32)
            fsbh = cpool.tile([128, 4], F32)
            nc.sync.dma_start(bm[:], d_bm[:])
            nc.sync.dma_start(fubh[:], d_fubh[:])
            nc.sync.dma_start(fsbh[:], d_fsbh[:])
            ones = cpool.tile([128, 128], F32)
            nc.vector.memset(ones[:], 1.0)

            for s in range(2):
                Q = QT[s]
                T = TK[s]
                F = Q * 128

                audioT = apool.tile([128, 4, LQ], F32, tag="audioT")
                for dt in range(4):
                    nc.sync.dma_start(audioT[:, dt, :], d_audioT[s, bass.ts(dt, 128), :])
                text = apool.tile([LKV, D], F32, tag="text")
                nc.sync.dma_start(text[:], d_text[s])
                textT = apool.tile([128, 4, LKV], F32, tag="textT")
                for dt in range(4):
                    nc.sync.dma_start(textT[:, dt, :], d_textT[s, bass.ts(dt, 128), :])
                wm = apool.tile([LKV, LQ], mybir.dt.int32, tag="wm")
                am = apool.tile([LKV, LQ], F32, tag="am")
                nc.sync.dma_start(wm[:], d_wm[s])
                nc.sync.dma_start(am[:], d_am[s])

                # ---- qa[a,q] (4 a-tiles) and kb[a,k] ----
                qa = apool.tile([128, 4, LQ], F32, tag="qa")
                kb = apool.tile([128, 4, LKV], F32, tag="kb")
                for at in range(4):
                    qps = ppool.tile([128, 512], F32, tag="mm")
                    for dt in range(4):
                        _mm(nc, qps[:, 0:F], wq[:, dt, bass.ts(at, 128)], audioT[:, dt, 0:F],
                            start=(dt == 0), stop=(dt == 3))
                    nc.vector.tensor_copy(qa[:, at, 0:F], qps[:, 0:F])
                    kps = ppool.tile([128, LKV], F32, tag="mm")
                    for dt in range(4):
                        nc.tensor.matmul(kps[:], wkv[:, dt, bass.ts(at, 128)], textT[:, dt, :],
                                         start=(dt == 0), stop=(dt == 3))
                    # kb = kps + h2attn_b[a]  (fused with PSUM->SBUF copy)
                    nc.vector.tensor_scalar_add(kb[:, at, :], kps[:], bm[:, at:at + 1])

                # ---- score[k,q] via tanh + v (x) e_k matmuls, all into one [32,512] PSUM ----
                sps = scps.tile([LKV, 512], F32, tag="sc")
                part = T < LKV or F < 512
                if part:  # zero-init full bank so unwritten rows/cols are finite
                    nc.tensor.matmul(sps[:], zw[:], zrow[:], start=True, stop=False,
                                     skip_group_check=True)
                for k in range(T):
                    h = hpool.tile([128, 4, F], BF16, tag="H")
                    for at in range(4):
                        nc.scalar.activation(h[:, at, :], qa[:, at, 0:F], AF.Tanh,
                                             bias=kb[:, at, k:k + 1])
                    for at in range(4):
                        for hl in range(2):
                            nc.tensor.matmul(
                                sps[:, 0:F],
                                vke[:, at, k, hl, :],
                                h[:, at, :],
                                start=(not part and k == 0 and at == 0 and hl == 0),
                                stop=(k == T - 1 and at == 3 and hl == 1),
                                skip_group_check=True)

                # ---- E' = select(W, exp(score), A) ----
                ex = apool.tile([LKV, LQ], F32, tag="ex")
                nc.scalar.activation(ex[:], sps[:], AF.Exp)
                ep = apool.tile([LKV, LQ], F32, tag="ep")
                nc.vector.select(ep[:], wm[:], ex[:], am[:])

                # ---- denom (broadcast to 128 partitions) + reciprocal ----
                dps = ppool.tile([128, 512], F32, tag="mm")
                nc.tensor.matmul(dps[:], ones[0:LKV, :], ep[:], start=True, stop=True)
                rden = apool.tile([128, LQ], F32, tag="rden")
                nc.vector.reciprocal(rden[:], dps[:])

                # ---- snippet^T[d,q], normalized by 1/denom during PSUM->SBUF move ----
                snip = apool.tile([128, 4, LQ], F32, tag="snip")
                for dt in range(4):
                    sps2 = ppool.tile([128, 512], F32, tag="mm")
                    _mm(nc, sps2[:], text[:, bass.ts(dt, 128)], ep[:], start=True, stop=True)
                    nc.vector.tensor_mul(snip[:, dt, :], sps2[:], rden[:])

                # ---- g_u^T = sigmoid(fuw^T audio + bu) as 0.5*tanh(0.5x+bu/2)+0.5 ----
                gu = apool.tile([128, 4, LQ], F32, tag="gu")
                gs = apool.tile([128, 4, LQ], F32, tag="gs")
                for et in range(4):
                    gps = ppool.tile([128, 512], F32, tag="mm")
                    for dt in range(4):
                        _mm(nc, gps[:], fuwT[:, dt, bass.ts(et, 128)], audioT[:, dt, :],
                            start=(dt == 0), stop=(dt == 3))
                    nc.scalar.activation(gu[:, et, :], gps[:], AF.Tanh,
                                         bias=fubh[:, et:et + 1], scale=0.5)
                    nc.vector.tensor_scalar(gu[:, et, :], gu[:, et, :], 0.5, 0.5,
                                            op0=mybir.AluOpType.mult, op1=mybir.AluOpType.add)
                    zps = ppool.tile([128, 512], F32, tag="mm")
                    for dt in range(4):
                        _mm(nc, zps[:], fswT[:, dt, bass.ts(et, 128)], snip[:, dt, :],
                            start=(dt == 0), stop=(dt == 3))
                    nc.scalar.activation(gs[:, et, :], zps[:], AF.Tanh,
                                         bias=fsbh[:, et:et + 1], scale=0.5)
                    nc.vector.tensor_scalar(gs[:, et, :], gs[:, et, :], 0.5, 0.5,
                                            op0=mybir.AluOpType.mult, op1=mybir.AluOpType.add)

                # ---- u, s, and the three reductions ----
                ups = redps.tile([1, 512], F32, tag="uu")
                sps3 = redps.tile([1, 512], F32, tag="ss")
                xps = redps.tile([1, 512], F32, tag="us")
                for dt in range(4):
                    u = hpool.tile([128, LQ], F32, tag="u")
                    sv = hpool.tile([128, LQ], F32, tag="sv")
                    nc.vector.tensor_mul(u[:], audioT[:, dt, :], gs[:, dt, :])
                    nc.vector.tensor_mul(sv[:], snip[:, dt, :], gu[:, dt, :])
                    t2 = hpool.tile([128, LQ], F32, tag="t2")
                    nc.vector.tensor_mul(t2[:], u[:], u[:])
                    _mm(nc, ups[:], ones[:, 0:1], t2[:], start=(dt == 0), stop=(dt == 3))
                    t3 = hpool.tile([128, LQ], F32, tag="t3")
                    nc.vector.tensor_mul(t3[:], sv[:], sv[:])
                    _mm(nc, sps3[:], ones[:, 0:1], t3[:], start=(dt == 0), stop=(dt == 3))
                    t4 = hpool.tile([128, LQ], F32, tag="t4")
                    nc.vector.tensor_mul(t4[:], u[:], sv[:])
                    _mm(nc, xps[:], ones[:, 0:1], t4[:], start=(dt == 0), stop=(dt == 3))

                # ---- tail: out = exp(-sqrt(2-2*us/sqrt(uu*ss))) ----
                uu = spool.tile([1, 512], F32, tag="uu_s")
                ss = spool.tile([1, 512], F32, tag="ss_s")
                us = spool.tile([1, 512], F32, tag="us_s")
                nc.vector.tensor_copy(uu[:], ups[:])
                nc.vector.tensor_copy(ss[:], sps3[:])
                nc.vector.tensor_copy(us[:], xps[:])
                p1 = spool.tile([1, 512], F32, tag="p1")
                nc.vector.tensor_mul(p1[:], uu[:], ss[:])
                nc.scalar.activation(p1[:], p1[:], AF.Ln)
                nc.scalar.activation(p1[:], p1[:], AF.Exp, scale=-0.5)  # rsqrt(uu*ss)
                tt = spool.tile([1, 512], F32, tag="tt")
                nc.vector.tensor_mul(tt[:], us[:], p1[:])               # cosine t
                nc.vector.tensor_scalar(tt[:], tt[:], -2.0, 2.0,
                                        op0=mybir.AluOpType.mult, op1=mybir.AluOpType.add)
                nc.scalar.activation(tt[:], tt[:], AF.Ln)               # ln(2-2t)
                nc.scalar.activation(tt[:], tt[:], AF.Exp, scale=0.5)   # sqrt(2-2t)
                nc.scalar.activation(tt[:], tt[:], AF.Exp, scale=-1.0)  # final
                nc.sync.dma_start(d_out[s:s + 1, :], tt[:])

    nc.compile()
    return nc


def kernel(**inputs):
    audio = np.ascontiguousarray(np.asarray(inputs["audio"], dtype=np.float32))
    text = np.ascontiguousarray(np.asarray(inputs["text"], dtype=np.float32))
    al = np.asarray(inputs["audio_len"]).astype(np.int64)
    tl = np.asarray(inputs["text_len"]).astype(np.int64)
    h2w = np.ascontiguousarray(np.asarray(inputs["h2attn_w"], dtype=np.float32))
    h2b = np.asarray(inputs["h2attn_b"], dtype=np.float32)
    v = np.asarray(inputs["v"], dtype=np.float32)
    fuw = np.asarray(inputs["fc_u_w"], dtype=np.float32)
    fub = np.asarray(inputs["fc_u_b"], dtype=np.float32)
    fsw = np.asarray(inputs["fc_s_w"], dtype=np.float32)
    fsb = np.asarray(inputs["fc_s_b"], dtype=np.float32)

    # slot assignment: 8 largest audio_len -> slot0
    order = np.argsort(-al, kind="stable")
    slots = [sorted(order[:NC].tolist()), sorted(order[NC:].tolist())]
    QT = [max(1, int(np.ceil(max(al[s_]) / 128))) for s_ in slots]
    TK = [max(1, int(max(tl[s_]))) for s_ in slots]

    nc = build_program(QT, TK)

    # host-side layout prep (transposes / reshapes only)
    audioT = np.ascontiguousarray(audio.transpose(0, 2, 1))     # [B, D, LQ]
    textT = np.ascontiguousarray(text.transpose(0, 2, 1))       # [B, D, LKV]
    fuwT = np.ascontiguousarray(fuw.T)
    fswT = np.ascontiguousarray(fsw.T)
    import ml_dtypes
    v_hi = v.astype(ml_dtypes.bfloat16).astype(np.float32)
    v_lo = v - v_hi
    vke = np.zeros((128, 4, LKV, 2, LKV), ml_dtypes.bfloat16)
    for k in range(LKV):
        vke[:, :, k, 0, k] = v_hi.reshape(4, 128).T
        vke[:, :, k, 1, k] = v_lo.reshape(4, 128).T
    bm = np.ascontiguousarray(h2b.reshape(4, 128).T)
    fubh = np.ascontiguousarray((fub * 0.5).reshape(4, 128).T)
    fsbh = np.ascontiguousarray((fsb * 0.5).reshape(4, 128).T)

    qi = np.arange(LQ)
    ki = np.arange(LKV)
    in_maps = []
    for c in range(NC):
        bsel = [slots[0][c], slots[1][c]]
        wms, ams = [], []
        for b in bsel:
            valid_q = qi < al[b]
            valid_k = ki < tl[b]
            wms.append((valid_k[:, None] & valid_q[None, :]).astype(np.int32))
            ams.append(np.broadcast_to((~valid_q)[None, :], (LKV, LQ)).astype(np.float32))
        in_maps.append({
            "audioT": np.ascontiguousarray(audioT[bsel]),
            "text": np.ascontiguousarray(text[bsel]),
            "textT": np.ascontiguousarray(textT[bsel]),
            "wmask": np.stack(wms), "amask": np.stack(ams),
            "h2w": h2w, "fuwT": fuwT, "fswT": fswT,
            "vke": vke, "bm": bm, "fubh": fubh, "fsbh": fsbh,
        })

    res = bass_utils.run_bass_kernel_spmd(nc, in_maps, core_ids=list(range(NC)))
    global LAST_RESULT
    LAST_RESULT = res
    import os as _os
    if _os.environ.get("KERNEL_TIME_RUNS"):
        import time as _t
        ts = []
        for _ in range(3):
            t0 = _t.perf_counter()
            bass_utils.run_bass_kernel_spmd(nc, in_maps, core_ids=list(range(NC)))
            ts.append(_t.perf_counter() - t0)
        print(f"HW exec time: {int(min(ts)*1e9)} ns  (best-of-3 full dispatch incl host transfer)")
    out = np.zeros((B, LQ), np.float32)
    for c in range(NC):
        for s in range(2):
            out[slots[s][c]] = res.results[c]["out"][s]
    return out


if __name__ == "__main__":
    d = np.load("/root/problem/inputs.npz")
    o = kernel(**{k: d[k] for k in d.files})
    print("out", o.shape, o.dtype, float(o.min()), float(o.max()))


# revision 22
# speedup vs baseline: 13631.9771x; 13631.9771x over previous
"""Trainium2 Bass kernel for AttnGatingExpL2 (additive attention + cross gating + exp-neg-L2).

Math (per batch b):
  qa[a,q]   = sum_d Wq[d,a] * audio[q,d]          (PE, audio^T staged on host)
  kb[a,k]   = sum_d Wkv[d,a] * text[k,d] + b[a]
  score[k,q]= sum_a v[a] * tanh(qa[a,q] + kb[a,k])  (ACT tanh w/ per-partition bias; PE matvec)
  E'        = select(W, exp(score), A)   -- reproduces reference masking exactly:
              valid (q<alen,k<tlen): exp(score); invalid k: 0; masked q: 1 for ALL k
              (=> attn uniform 1/32 over all 32 keys, as NEG-masked softmax gives)
  denom[q]  = sum_k E'[k,q]  (PE ones-matmul, broadcast to 128 partitions)
  snipT     = text^T @ E'    (unnormalized snippet, transposed world [d,q])
  g_uT      = sigmoid(fuw^T q-proj), g_sT = sigmoid((fsw^T snipT) * (1/denom))
  u=audioT*g_sT, s=snipT*g_uT (unnorm; denom cancels in cosine)
  out[q]    = exp(-sqrt(2-2*us/sqrt(uu*ss)))   (ln/exp form, division-free)

Sharding: data-parallel, 2 batches per core on 8 cores; batches sorted by audio_len
into slot0 (large, q-budget=ceil(max/128) tiles) and slot1 (small) so the tanh stage
only covers ceil(alen)x tlen budgets (trace-time constants; program is SPMD-identical).
"""

import numpy as np
from contextlib import ExitStack

import concourse.bass as bass
import concourse.tile as tile
from concourse import bacc, mybir
from concourse import bass_utils

F32 = mybir.dt.float32
F32R = mybir.dt.float32r
BF16 = mybir.dt.bfloat16
AF = mybir.ActivationFunctionType

NC = 8
B, LQ, LKV, D = 16, 512, 32, 512
NEGBIG = -1.0e10


def _mm(nc, out, lhsT, rhs, start, stop):
    nc.tensor.matmul(out, lhsT, rhs, start=start, stop=stop)


def build_program(QT, TK):
    """QT: [q-tiles slot0, slot1]; TK: [k-budget slot0, slot1]. Returns (nc, names)."""
    nc = bacc.Bacc("TRN2", target_bir_lowering=False, debug=False, num_devices=NC)

    # per-core DRAM inputs (values differ per core, names shared — SPMD)
    d_audioT = nc.dram_tensor("audioT", [2, D, LQ], F32R, kind="ExternalInput").ap()
    d_text = nc.dram_tensor("text", [2, LKV, D], F32, kind="ExternalInput").ap()
    d_textT = nc.dram_tensor("textT", [2, D, LKV], F32R, kind="ExternalInput").ap()
    d_wm = nc.dram_tensor("wmask", [2, LKV, LQ], mybir.dt.int32, kind="ExternalInput").ap()
    d_am = nc.dram_tensor("amask", [2, LKV, LQ], F32, kind="ExternalInput").ap()
    d_h2w = nc.dram_tensor("h2w", [D, 2 * D], F32R, kind="ExternalInput").ap()
    d_fuwT = nc.dram_tensor("fuwT", [D, D], F32R, kind="ExternalInput").ap()
    d_fswT = nc.dram_tensor("fswT", [D, D], F32R, kind="ExternalInput").ap()
    d_vke = nc.dram_tensor("vke", [128, 4, LKV, 2, LKV], BF16, kind="ExternalInput").ap()
    d_bm = nc.dram_tensor("bm", [128, 4], F32, kind="ExternalInput").ap()
    d_fubh = nc.dram_tensor("fubh", [128, 4], F32, kind="ExternalInput").ap()
    d_fsbh = nc.dram_tensor("fsbh", [128, 4], F32, kind="ExternalInput").ap()
    d_out = nc.dram_tensor("out", [2, LQ], F32, kind="ExternalOutput").ap()

    es = ExitStack()
    with tile.TileContext(nc) as tc:
        with es:
            cpool = es.enter_context(tc.tile_pool(name="consts", bufs=1))
            apool = es.enter_context(tc.tile_pool(name="acts", bufs=2))
            hpool = es.enter_context(tc.tile_pool(name="h", bufs=3))
            spool = es.enter_context(tc.tile_pool(name="small", bufs=1))
            ppool = es.enter_context(tc.tile_pool(name="ps", bufs=3, space=bass.MemorySpace.PSUM))
            scps = es.enter_context(tc.tile_pool(name="scps", bufs=2, space=bass.MemorySpace.PSUM))
            redps = es.enter_context(tc.tile_pool(name="redps", bufs=1, space=bass.MemorySpace.PSUM))

            # ---- constants (once per core) ----
            wq = cpool.tile([128, 4, D], F32R)   # [d_in_tile, dt, a]
            wkv = cpool.tile([128, 4, D], F32R)
            fuwT = cpool.tile([128, 4, D], F32R)  # [d, dt, e]
            fswT = cpool.tile([128, 4, D], F32R)
            for dt in range(4):
                nc.sync.dma_start(wq[:, dt, :], d_h2w[bass.ts(dt, 128), 0:D])
                nc.sync.dma_start(wkv[:, dt, :], d_h2w[bass.ts(dt, 128), D:2 * D])
            vke = cpool.tile([128, 4, LKV, 2, LKV], BF16)
            zrow = cpool.tile([1, 512], BF16)
            nc.vector.memset(zrow[:], 0.0)
            zw = cpool.tile([1, LKV], BF16)
            nc.vector.memset(zw[:], 0.0)
            bm = cpool.tile([128, 4], F32)
            fubh = cpool.tile([128, 4], F32)
            fsbh = cpool.tile([128, 4], F32)
            nc.sync.dma_start(bm[:], d_bm[:])
            nc.sync.dma_start(fubh[:], d_fubh[:])
            nc.sync.dma_start(fsbh[:], d_fsbh[:])
            ones = cpool.tile([128, 128], F32)
            nc.vector.memset(ones[:], 1.0)
            ones_bf = cpool.tile([128, 1], BF16)
            nc.vector.memset(ones_bf[:], 1.0)

            slot_tiles = {}
            for s in range(2):
                Q = QT[s]
                F = Q * 128
                audioT = apool.tile([128, 4, LQ], F32R, tag="audioT")
                for dt in range(4):
                    nc.sync.dma_start(audioT[:, dt, :], d_audioT[s, bass.ts(dt, 128), :])
                text = apool.tile([LKV, D], F32, tag="text")
                nc.sync.dma_start(text[:], d_text[s])
                textT = apool.tile([128, 4, LKV], F32R, tag="textT")
                for dt in range(4):
                    nc.sync.dma_start(textT[:, dt, :], d_textT[s, bass.ts(dt, 128), :])
                wm = apool.tile([LKV, LQ], mybir.dt.int32, tag="wm")
                am = apool.tile([LKV, LQ], F32, tag="am")
                nc.sync.dma_start(wm[:], d_wm[s])
                nc.sync.dma_start(am[:], d_am[s])

                qa = apool.tile([128, 4, LQ], F32, tag="qa")
                kb = apool.tile([128, 4, LKV], F32, tag="kb")
                for at in range(4):
                    qps = ppool.tile([128, 512], F32, tag="mm")
                    for dt in range(4):
                        _mm(nc, qps[:, 0:F], wq[:, dt, bass.ts(at, 128)], audioT[:, dt, 0:F],
                            start=(dt == 0), stop=(dt == 3))
                    nc.vector.tensor_copy(qa[:, at, 0:F], qps[:, 0:F])
                    kps = ppool.tile([128, LKV], F32, tag="mm")
                    for dt in range(4):
                        nc.tensor.matmul(kps[:], wkv[:, dt, bass.ts(at, 128)], textT[:, dt, :],
                                         start=(dt == 0), stop=(dt == 3))
                    nc.vector.tensor_scalar_add(kb[:, at, :], kps[:], bm[:, at:at + 1])
                slot_tiles[s] = (audioT, text, wm, am, qa, kb)
                if s == 0:  # vke feeds the first matvecs ~7us in; load before slot1 staging
                    nc.sync.dma_start(vke[:], d_vke[:])

            # deferred non-critical weight loads (after first-tanh dependencies)
            for dt in range(4):
                nc.sync.dma_start(fuwT[:, dt, :], d_fuwT[bass.ts(dt, 128), :])
                nc.sync.dma_start(fswT[:, dt, :], d_fswT[bass.ts(dt, 128), :])

            for s in range(2):
                Q = QT[s]
                T = TK[s]
                F = Q * 128
                audioT, text, wm, am, qa, kb = slot_tiles[s]

                # ---- score[k,q] via tanh + v (x) e_k matmuls, all into one [32,512] PSUM ----
                sps = scps.tile([LKV, 512], F32, tag="sc")
                part = T < LKV or F < 512
                if part:  # zero-init full bank so unwritten rows/cols are finite
                    nc.tensor.matmul(sps[:], zw[:], zrow[:], start=True, stop=False,
                                     skip_group_check=True)
                for k in range(T):
                    h = hpool.tile([128, 4, F], BF16, tag="H")
                    for at in range(4):
                        nc.scalar.activation(h[:, at, :], qa[:, at, 0:F], AF.Tanh,
                                             bias=kb[:, at, k:k + 1])
                    for at in range(4):
                        for hl in range(2):
                            nc.tensor.matmul(
                                sps[:, 0:F],
                                vke[:, at, k, hl, :],
                                h[:, at, :],
                                start=(not part and k == 0 and at == 0 and hl == 0),
                                stop=(k == T - 1 and at == 3 and hl == 1),
                                skip_group_check=True)

                # ---- E' = select(W, exp(score), A) ----
                ex = apool.tile([LKV, LQ], F32, tag="ex")
                nc.scalar.activation(ex[:], sps[:], AF.Exp)
                ep = apool.tile([LKV, LQ], F32, tag="ep")
                nc.vector.select(ep[:], wm[:], ex[:], am[:])

                # ---- denom (broadcast to 128 partitions) + reciprocal ----
                dps = ppool.tile([128, 512], F32, tag="mm")
                nc.tensor.matmul(dps[:], ones[0:LKV, :], ep[:], start=True, stop=True)
                rden = apool.tile([128, LQ], F32, tag="rden")
                nc.vector.reciprocal(rden[:], dps[:])

                # ---- snippet^T[d,q], normalized by 1/denom during PSUM->SBUF move ----
                snip = apool.tile([128, 4, LQ], F32R, tag="snip")
                for dt in range(4):
                    sps2 = ppool.tile([128, 512], F32, tag="mm")
                    _mm(nc, sps2[:], text[:, bass.ts(dt, 128)], ep[:], start=True, stop=True)
                    nc.vector.tensor_mul(snip[:, dt, :], sps2[:], rden[:])

                # ---- g_u^T = sigmoid(fuw^T audio + bu) as 0.5*tanh(0.5x+bu/2)+0.5 ----
                gu = apool.tile([128, 4, LQ], F32, tag="gu")
                gs = apool.tile([128, 4, LQ], F32, tag="gs")
                for et in range(4):
                    gps = ppool.tile([128, 512], F32, tag="mm")
                    for dt in range(4):
                        _mm(nc, gps[:], fuwT[:, dt, bass.ts(et, 128)], audioT[:, dt, :],
                            start=(dt == 0), stop=(dt == 3))
                    nc.scalar.activation(gu[:, et, :], gps[:], AF.Tanh,
                                         bias=fubh[:, et:et + 1], scale=0.5)
                    nc.vector.tensor_scalar(gu[:, et, :], gu[:, et, :], 0.5, 0.5,
                                            op0=mybir.AluOpType.mult, op1=mybir.AluOpType.add)
                    zps = ppool.tile([128, 512], F32, tag="mm")
                    for dt in range(4):
                        _mm(nc, zps[:], fswT[:, dt, bass.ts(et, 128)], snip[:, dt, :],
                            start=(dt == 0), stop=(dt == 3))
                    nc.scalar.activation(gs[:, et, :], zps[:], AF.Tanh,
                                         bias=fsbh[:, et:et + 1], scale=0.5)
                    nc.vector.tensor_scalar(gs[:, et, :], gs[:, et, :], 0.5, 0.5,
                                            op0=mybir.AluOpType.mult, op1=mybir.AluOpType.add)

                # ---- u, s, and the three reductions ----
                ups = redps.tile([1, 512], F32, tag="uu")
                sps3 = redps.tile([1, 512], F32, tag="ss")
                xps = redps.tile([1, 512], F32, tag="us")
                for dt in range(4):
                    u = hpool.tile([128, LQ], F32, tag="u")
                    sv = hpool.tile([128, LQ], F32, tag="sv")
                    nc.vector.tensor_mul(u[:], audioT[:, dt, :], gs[:, dt, :])
                    nc.vector.tensor_mul(sv[:], snip[:, dt, :], gu[:, dt, :])
                    t2 = hpool.tile([128, LQ], BF16, tag="t2")
                    nc.vector.tensor_mul(t2[:], u[:], u[:])
                    _mm(nc, ups[:], ones_bf[:], t2[:], start=(dt == 0), stop=(dt == 3))
                    t3 = hpool.tile([128, LQ], BF16, tag="t3")
                    nc.vector.tensor_mul(t3[:], sv[:], sv[:])
                    _mm(nc, sps3[:], ones_bf[:], t3[:], start=(dt == 0), stop=(dt == 3))
                    t4 = hpool.tile([128, LQ], BF16, tag="t4")
                    nc.vector.tensor_mul(t4[:], u[:], sv[:])
                    _mm(nc, xps[:], ones_bf[:], t4[:], start=(dt == 0), stop=(dt == 3))

                # ---- tail: out = exp(-sqrt(2-2*us/sqrt(uu*ss))) ----
                uu = spool.tile([1, 512], F32, tag="uu_s")
                ss = spool.tile([1, 512], F32, tag="ss_s")
                us = spool.tile([1, 512], F32, tag="us_s")
                nc.vector.tensor_copy(uu[:], ups[:])
                nc.vector.tensor_copy(ss[:], sps3[:])
                nc.vector.tensor_copy(us[:], xps[:])
                p1 = spool.tile([1, 512], F32, tag="p1")
                nc.vector.tensor_mul(p1[:], uu[:], ss[:])
                nc.scalar.activation(p1[:], p1[:], AF.Ln)
                nc.scalar.activation(p1[:], p1[:], AF.Exp, scale=-0.5)  # rsqrt(uu*ss)
                tt = spool.tile([1, 512], F32, tag="tt")
                nc.vector.tensor_mul(tt[:], us[:], p1[:])               # cosine t
                nc.vector.tensor_scalar(tt[:], tt[:], -2.0, 2.0,
                                        op0=mybir.AluOpType.mult, op1=mybir.AluOpType.add)
                nc.scalar.activation(tt[:], tt[:], AF.Ln)               # ln(2-2t)
                nc.scalar.activation(tt[:], tt[:], AF.Exp, scale=0.5)   # sqrt(2-2t)
                nc.scalar.activation(tt[:], tt[:], AF.Exp, scale=-1.0)  # final
                nc.sync.dma_start(d_out[s:s + 1, :], tt[:])

    nc.compile()
    return nc


def kernel(**inputs):
    audio = np.ascontiguousarray(np.asarray(inputs["audio"], dtype=np.float32))
    text = np.ascontiguousarray(np.asarray(inputs["text"], dtype=np.float32))
    al = np.asarray(inputs["audio_len"]).astype(np.int64)
    tl = np.asarray(inputs["text_len"]).astype(np.int64)
    h2w = np.ascontiguousarray(np.asarray(inputs["h2attn_w"], dtype=np.float32))
    h2b = np.asarray(inputs["h2attn_b"], dtype=np.float32)
    v = np.asarray(inputs["v"], dtype=np.float32)
    fuw = np.asarray(inputs["fc_u_w"], dtype=np.float32)
    fub = np.asarray(inputs["fc_u_b"], dtype=np.float32)
    fsw = np.asarray(inputs["fc_s_w"], dtype=np.float32)
    fsb = np.asarray(inputs["fc_s_b"], dtype=np.float32)

    # slot assignment: 8 largest audio_len -> slot0
    order = np.argsort(-al, kind="stable")
    slots = [sorted(order[:NC].tolist()), sorted(order[NC:].tolist())]
    QT = [max(1, int(np.ceil(max(al[s_]) / 128))) for s_ in slots]
    TK = [max(1, int(max(tl[s_]))) for s_ in slots]

    nc = build_program(QT, TK)

    # host-side layout prep (transposes / reshapes only)
    audioT = np.ascontiguousarray(audio.transpose(0, 2, 1))     # [B, D, LQ]
    textT = np.ascontiguousarray(text.transpose(0, 2, 1))       # [B, D, LKV]
    fuwT = np.ascontiguousarray(fuw.T)
    fswT = np.ascontiguousarray(fsw.T)
    import ml_dtypes
    v_hi = v.astype(ml_dtypes.bfloat16).astype(np.float32)
    v_lo = v - v_hi
    vke = np.zeros((128, 4, LKV, 2, LKV), ml_dtypes.bfloat16)
    for k in range(LKV):
        vke[:, :, k, 0, k] = v_hi.reshape(4, 128).T
        vke[:, :, k, 1, k] = v_lo.reshape(4, 128).T
    bm = np.ascontiguousarray(h2b.reshape(4, 128).T)
    fubh = np.ascontiguousarray((fub * 0.5).reshape(4, 128).T)
    fsbh = np.ascontiguousarray((fsb * 0.5).reshape(4, 128).T)

    qi = np.arange(LQ)
    ki = np.arange(LKV)
    in_maps = []
    for c in range(NC):
        bsel = [slots[0][c], slots[1][c]]
        wms, ams = [], []
        for b in bsel:
            valid_q = qi < al[b]
            valid_k = ki < tl[b]
            wms.append((valid_k[:, None] & valid_q[None, :]).astype(np.int32))
            ams.append(np.broadcast_to((~valid_q)[None, :], (LKV, LQ)).astype(np.float32))
        in_maps.append({
            "audioT": np.ascontiguousarray(audioT[bsel]),
            "text": np.ascontiguousarray(text[bsel]),
            "textT": np.ascontiguousarray(textT[bsel]),
            "wmask": np.stack(wms), "amask": np.stack(ams),
            "h2w": h2w, "fuwT": fuwT, "fswT": fswT,
            "vke": vke, "bm": bm, "fubh": fubh, "fsbh": fsbh,
        })

    res = bass_utils.run_bass_kernel_spmd(nc, in_maps, core_ids=list(range(NC)))
    global LAST_RESULT
    LAST_RESULT = res
    import os as _os
    if _os.environ.get("KERNEL_TIME_RUNS"):
        import time as _t
        ts = []
        for _ in range(3):
            t0 = _t.perf_counter()
            bass_utils.run_bass_kernel_spmd(nc, in_maps, core_ids=list(range(NC)))
            ts.append(_t.perf_counter() - t0)
        print(f"HW exec time: {int(min(ts)*1e9)} ns  (best-of-3 full dispatch incl host transfer)")
    out = np.zeros((B, LQ), np.float32)
    for c in range(NC):
        for s in range(2):
            out[slots[s][c]] = res.results[c]["out"][s]
    return out


if __name__ == "__main__":
    d = np.load("/root/problem/inputs.npz")
    o = kernel(**{k: d[k] for k in d.files})
    print("out", o.shape, o.dtype, float(o.min()), float(o.max()))


# revision 23
# speedup vs baseline: 13720.2909x; 1.0065x over previous
"""Trainium2 Bass kernel for AttnGatingExpL2 (additive attention + cross gating + exp-neg-L2).

Math (per batch b):
  qa[a,q]   = sum_d Wq[d,a] * audio[q,d]          (PE, audio^T staged on host)
  kb[a,k]   = sum_d Wkv[d,a] * text[k,d] + b[a]
  score[k,q]= sum_a v[a] * tanh(qa[a,q] + kb[a,k])  (ACT tanh w/ per-partition bias; PE matvec)
  E'        = select(W, exp(score), A)   -- reproduces reference masking exactly:
              valid (q<alen,k<tlen): exp(score); invalid k: 0; masked q: 1 for ALL k
              (=> attn uniform 1/32 over all 32 keys, as NEG-masked softmax gives)
  denom[q]  = sum_k E'[k,q]  (PE ones-matmul, broadcast to 128 partitions)
  snipT     = text^T @ E'    (unnormalized snippet, transposed world [d,q])
  g_uT      = sigmoid(fuw^T q-proj), g_sT = sigmoid((fsw^T snipT) * (1/denom))
  u=audioT*g_sT, s=snipT*g_uT (unnorm; denom cancels in cosine)
  out[q]    = exp(-sqrt(2-2*us/sqrt(uu*ss)))   (ln/exp form, division-free)

Sharding: data-parallel, 2 batches per core on 8 cores; batches sorted by audio_len
into slot0 (large, q-budget=ceil(max/128) tiles) and slot1 (small) so the tanh stage
only covers ceil(alen)x tlen budgets (trace-time constants; program is SPMD-identical).
"""

import numpy as np
from contextlib import ExitStack

import concourse.bass as bass
import concourse.tile as tile
from concourse import bacc, mybir
from concourse import bass_utils

F32 = mybir.dt.float32
F32R = mybir.dt.float32r
BF16 = mybir.dt.bfloat16
AF = mybir.ActivationFunctionType

NC = 8
B, LQ, LKV, D = 16, 512, 32, 512
NEGBIG = -1.0e10


def _mm(nc, out, lhsT, rhs, start, stop):
    nc.tensor.matmul(out, lhsT, rhs, start=start, stop=stop)


def build_program(QT, TK):
    """QT: [q-tiles slot0, slot1]; TK: [k-budget slot0, slot1]. Returns (nc, names)."""
    nc = bacc.Bacc("TRN2", target_bir_lowering=False, debug=False, num_devices=NC)

    # per-core DRAM inputs (values differ per core, names shared — SPMD)
    d_audioT = nc.dram_tensor("audioT", [2, D, LQ], F32R, kind="ExternalInput").ap()
    d_text = nc.dram_tensor("text", [2, LKV, D], F32, kind="ExternalInput").ap()
    d_textT = nc.dram_tensor("textT", [2, D, LKV], F32R, kind="ExternalInput").ap()
    d_wm = nc.dram_tensor("wmask", [2, LKV, LQ], mybir.dt.int32, kind="ExternalInput").ap()
    d_am = nc.dram_tensor("amask", [2, LKV, LQ], F32, kind="ExternalInput").ap()
    d_h2w = nc.dram_tensor("h2w", [D, 2 * D], F32R, kind="ExternalInput").ap()
    d_fuwT = nc.dram_tensor("fuwT", [D, D], F32R, kind="ExternalInput").ap()
    d_fswT = nc.dram_tensor("fswT", [D, D], F32R, kind="ExternalInput").ap()
    d_vke = nc.dram_tensor("vke", [128, 4, LKV, 2, LKV], BF16, kind="ExternalInput").ap()
    d_bm = nc.dram_tensor("bm", [128, 4], F32, kind="ExternalInput").ap()
    d_fubh = nc.dram_tensor("fubh", [128, 4], F32, kind="ExternalInput").ap()
    d_fsbh = nc.dram_tensor("fsbh", [128, 4], F32, kind="ExternalInput").ap()
    d_out = nc.dram_tensor("out", [2, LQ], F32, kind="ExternalOutput").ap()

    es = ExitStack()
    with tile.TileContext(nc) as tc:
        with es:
            cpool = es.enter_context(tc.tile_pool(name="consts", bufs=1))
            apool = es.enter_context(tc.tile_pool(name="acts", bufs=2))
            hpool = es.enter_context(tc.tile_pool(name="h", bufs=3))
            spool = es.enter_context(tc.tile_pool(name="small", bufs=1))
            ppool = es.enter_context(tc.tile_pool(name="ps", bufs=3, space=bass.MemorySpace.PSUM))
            scps = es.enter_context(tc.tile_pool(name="scps", bufs=2, space=bass.MemorySpace.PSUM))
            redps = es.enter_context(tc.tile_pool(name="redps", bufs=1, space=bass.MemorySpace.PSUM))

            # ---- constants (once per core) ----
            wq = cpool.tile([128, 4, D], F32R)   # [d_in_tile, dt, a]
            wkv = cpool.tile([128, 4, D], F32R)
            fuwT = cpool.tile([128, 4, D], F32R)  # [d, dt, e]
            fswT = cpool.tile([128, 4, D], F32R)
            for dt in range(4):
                nc.sync.dma_start(wq[:, dt, :], d_h2w[bass.ts(dt, 128), 0:D])
                nc.sync.dma_start(wkv[:, dt, :], d_h2w[bass.ts(dt, 128), D:2 * D])
            vke = cpool.tile([128, 4, LKV, 2, LKV], BF16)
            zrow = cpool.tile([1, 512], BF16)
            nc.vector.memset(zrow[:], 0.0)
            zw = cpool.tile([1, LKV], BF16)
            nc.vector.memset(zw[:], 0.0)
            bm = cpool.tile([128, 4], F32)
            fubh = cpool.tile([128, 4], F32)
            fsbh = cpool.tile([128, 4], F32)
            nc.sync.dma_start(bm[:], d_bm[:])
            nc.sync.dma_start(fubh[:], d_fubh[:])
            nc.sync.dma_start(fsbh[:], d_fsbh[:])
            ones = cpool.tile([128, 128], F32)
            nc.vector.memset(ones[:], 1.0)
            ones_bf = cpool.tile([128, 1], BF16)
            nc.vector.memset(ones_bf[:], 1.0)

            slot_tiles = {}
            for s in range(2):
                Q = QT[s]
                F = Q * 128
                audioT = apool.tile([128, 4, LQ], F32R, tag="audioT")
                for dt in range(4):
                    nc.sync.dma_start(audioT[:, dt, :], d_audioT[s, bass.ts(dt, 128), :])
                text = apool.tile([LKV, D], F32, tag="text")
                nc.sync.dma_start(text[:], d_text[s])
                textT = apool.tile([128, 4, LKV], F32R, tag="textT")
                for dt in range(4):
                    nc.sync.dma_start(textT[:, dt, :], d_textT[s, bass.ts(dt, 128), :])
                wm = apool.tile([LKV, LQ], mybir.dt.int32, tag="wm")
                am = apool.tile([LKV, LQ], F32, tag="am")

                qa = apool.tile([128, 4, LQ], F32, tag="qa")
                kb = apool.tile([128, 4, LKV], F32, tag="kb")
                for at in range(4):
                    qps = ppool.tile([128, 512], F32, tag="mm")
                    for dt in range(4):
                        _mm(nc, qps[:, 0:F], wq[:, dt, bass.ts(at, 128)], audioT[:, dt, 0:F],
                            start=(dt == 0), stop=(dt == 3))
                    nc.vector.tensor_copy(qa[:, at, 0:F], qps[:, 0:F])
                    kps = ppool.tile([128, LKV], F32, tag="mm")
                    for dt in range(4):
                        nc.tensor.matmul(kps[:], wkv[:, dt, bass.ts(at, 128)], textT[:, dt, :],
                                         start=(dt == 0), stop=(dt == 3))
                    nc.vector.tensor_scalar_add(kb[:, at, :], kps[:], bm[:, at:at + 1])
                slot_tiles[s] = (audioT, text, wm, am, qa, kb)
                if s == 0:  # vke feeds the first matvecs ~7us in; load before slot1 staging
                    nc.sync.dma_start(vke[:], d_vke[:])

            # deferred non-critical loads (not needed until after the k-loops)
            for s in range(2):
                _, _, wm_s, am_s, _, _ = slot_tiles[s]
                nc.sync.dma_start(wm_s[:], d_wm[s])
                nc.sync.dma_start(am_s[:], d_am[s])
            for dt in range(4):
                nc.sync.dma_start(fuwT[:, dt, :], d_fuwT[bass.ts(dt, 128), :])
                nc.sync.dma_start(fswT[:, dt, :], d_fswT[bass.ts(dt, 128), :])

            for s in range(2):
                Q = QT[s]
                T = TK[s]
                F = Q * 128
                audioT, text, wm, am, qa, kb = slot_tiles[s]

                # ---- score[k,q] via tanh + v (x) e_k matmuls, all into one [32,512] PSUM ----
                sps = scps.tile([LKV, 512], F32, tag="sc")
                part = T < LKV or F < 512
                if part:  # zero-init full bank so unwritten rows/cols are finite
                    nc.tensor.matmul(sps[:], zw[:], zrow[:], start=True, stop=False,
                                     skip_group_check=True)
                for k in range(T):
                    h = hpool.tile([128, 4, F], BF16, tag="H")
                    for at in range(4):
                        nc.scalar.activation(h[:, at, :], qa[:, at, 0:F], AF.Tanh,
                                             bias=kb[:, at, k:k + 1])
                    for at in range(4):
                        for hl in range(2):
                            nc.tensor.matmul(
                                sps[:, 0:F],
                                vke[:, at, k, hl, :],
                                h[:, at, :],
                                start=(not part and k == 0 and at == 0 and hl == 0),
                                stop=(k == T - 1 and at == 3 and hl == 1),
                                skip_group_check=True)

                # ---- E' = select(W, exp(score), A) ----
                ex = apool.tile([LKV, LQ], F32, tag="ex")
                nc.scalar.activation(ex[:], sps[:], AF.Exp)
                ep = apool.tile([LKV, LQ], F32, tag="ep")
                nc.vector.select(ep[:], wm[:], ex[:], am[:])

                # ---- denom (broadcast to 128 partitions) + reciprocal ----
                dps = ppool.tile([128, 512], F32, tag="mm")
                nc.tensor.matmul(dps[:], ones[0:LKV, :], ep[:], start=True, stop=True)
                rden = apool.tile([128, LQ], F32, tag="rden")
                nc.vector.reciprocal(rden[:], dps[:])

                # ---- snippet^T[d,q], normalized by 1/denom during PSUM->SBUF move ----
                snip = apool.tile([128, 4, LQ], F32R, tag="snip")
                for dt in range(4):
                    sps2 = ppool.tile([128, 512], F32, tag="mm")
                    _mm(nc, sps2[:], text[:, bass.ts(dt, 128)], ep[:], start=True, stop=True)
                    nc.vector.tensor_mul(snip[:, dt, :], sps2[:], rden[:])

                # ---- g_u^T = sigmoid(fuw^T audio + bu) as 0.5*tanh(0.5x+bu/2)+0.5 ----
                gu = apool.tile([128, 4, LQ], F32, tag="gu")
                gs = apool.tile([128, 4, LQ], F32, tag="gs")
                for et in range(4):
                    gps = ppool.tile([128, 512], F32, tag="mm")
                    for dt in range(4):
                        _mm(nc, gps[:], fuwT[:, dt, bass.ts(et, 128)], audioT[:, dt, :],
                            start=(dt == 0), stop=(dt == 3))
                    nc.scalar.activation(gu[:, et, :], gps[:], AF.Tanh,
                                         bias=fubh[:, et:et + 1], scale=0.5)
                    nc.vector.tensor_scalar(gu[:, et, :], gu[:, et, :], 0.5, 0.5,
                                            op0=mybir.AluOpType.mult, op1=mybir.AluOpType.add)
                    zps = ppool.tile([128, 512], F32, tag="mm")
                    for dt in range(4):
                        _mm(nc, zps[:], fswT[:, dt, bass.ts(et, 128)], snip[:, dt, :],
                            start=(dt == 0), stop=(dt == 3))
                    nc.scalar.activation(gs[:, et, :], zps[:], AF.Tanh,
                                         bias=fsbh[:, et:et + 1], scale=0.5)
                    nc.vector.tensor_scalar(gs[:, et, :], gs[:, et, :], 0.5, 0.5,
                                            op0=mybir.AluOpType.mult, op1=mybir.AluOpType.add)

                # ---- u, s, and the three reductions ----
                ups = redps.tile([1, 512], F32, tag="uu")
                sps3 = redps.tile([1, 512], F32, tag="ss")
                xps = redps.tile([1, 512], F32, tag="us")
                for dt in range(4):
                    u = hpool.tile([128, LQ], F32, tag="u")
                    sv = hpool.tile([128, LQ], F32, tag="sv")
                    nc.vector.tensor_mul(u[:], audioT[:, dt, :], gs[:, dt, :])
                    nc.vector.tensor_mul(sv[:], snip[:, dt, :], gu[:, dt, :])
                    t2 = hpool.tile([128, LQ], BF16, tag="t2")
                    nc.vector.tensor_mul(t2[:], u[:], u[:])
                    _mm(nc, ups[:], ones_bf[:], t2[:], start=(dt == 0), stop=(dt == 3))
                    t3 = hpool.tile([128, LQ], BF16, tag="t3")
                    nc.vector.tensor_mul(t3[:], sv[:], sv[:])
                    _mm(nc, sps3[:], ones_bf[:], t3[:], start=(dt == 0), stop=(dt == 3))
                    t4 = hpool.tile([128, LQ], BF16, tag="t4")
                    nc.vector.tensor_mul(t4[:], u[:], sv[:])
                    _mm(nc, xps[:], ones_bf[:], t4[:], start=(dt == 0), stop=(dt == 3))

                # ---- tail: out = exp(-sqrt(2-2*us/sqrt(uu*ss))) ----
                uu = spool.tile([1, 512], F32, tag="uu_s")
                ss = spool.tile([1, 512], F32, tag="ss_s")
                us = spool.tile([1, 512], F32, tag="us_s")
                nc.vector.tensor_copy(uu[:], ups[:])
                nc.vector.tensor_copy(ss[:], sps3[:])
                nc.vector.tensor_copy(us[:], xps[:])
                p1 = spool.tile([1, 512], F32, tag="p1")
                nc.vector.tensor_mul(p1[:], uu[:], ss[:])
                nc.scalar.activation(p1[:], p1[:], AF.Ln)
                nc.scalar.activation(p1[:], p1[:], AF.Exp, scale=-0.5)  # rsqrt(uu*ss)
                tt = spool.tile([1, 512], F32, tag="tt")
                nc.vector.tensor_mul(tt[:], us[:], p1[:])               # cosine t
                nc.vector.tensor_scalar(tt[:], tt[:], -2.0, 2.0,
                                        op0=mybir.AluOpType.mult, op1=mybir.AluOpType.add)
                nc.scalar.activation(tt[:], tt[:], AF.Ln)               # ln(2-2t)
                nc.scalar.activation(tt[:], tt[:], AF.Exp, scale=0.5)   # sqrt(2-2t)
                nc.scalar.activation(tt[:], tt[:], AF.Exp, scale=-1.0)  # final
                nc.sync.dma_start(d_out[s:s + 1, :], tt[:])

    nc.compile()
    return nc


def kernel(**inputs):
    audio = np.ascontiguousarray(np.asarray(inputs["audio"], dtype=np.float32))
    text = np.ascontiguousarray(np.asarray(inputs["text"], dtype=np.float32))
    al = np.asarray(inputs["audio_len"]).astype(np.int64)
    tl = np.asarray(inputs["text_len"]).astype(np.int64)
    h2w = np.ascontiguousarray(np.asarray(inputs["h2attn_w"], dtype=np.float32))
    h2b = np.asarray(inputs["h2attn_b"], dtype=np.float32)
    v = np.asarray(inputs["v"], dtype=np.float32)
    fuw = np.asarray(inputs["fc_u_w"], dtype=np.float32)
    fub = np.asarray(inputs["fc_u_b"], dtype=np.float32)
    fsw = np.asarray(inputs["fc_s_w"], dtype=np.float32)
    fsb = np.asarray(inputs["fc_s_b"], dtype=np.float32)

    # slot assignment: 8 largest audio_len -> slot0
    order = np.argsort(-al, kind="stable")
    slots = [sorted(order[:NC].tolist()), sorted(order[NC:].tolist())]
    QT = [max(1, int(np.ceil(max(al[s_]) / 128))) for s_ in slots]
    TK = [max(1, int(max(tl[s_]))) for s_ in slots]

    nc = build_program(QT, TK)

    # host-side layout prep (transposes / reshapes only)
    audioT = np.ascontiguousarray(audio.transpose(0, 2, 1))     # [B, D, LQ]
    textT = np.ascontiguousarray(text.transpose(0, 2, 1))       # [B, D, LKV]
    fuwT = np.ascontiguousarray(fuw.T)
    fswT = np.ascontiguousarray(fsw.T)
    import ml_dtypes
    v_hi = v.astype(ml_dtypes.bfloat16).astype(np.float32)
    v_lo = v - v_hi
    vke = np.zeros((128, 4, LKV, 2, LKV), ml_dtypes.bfloat16)
    for k in range(LKV):
        vke[:, :, k, 0, k] = v_hi.reshape(4, 128).T
        vke[:, :, k, 1, k] = v_lo.reshape(4, 128).T
    bm = np.ascontiguousarray(h2b.reshape(4, 128).T)
    fubh = np.ascontiguousarray((fub * 0.5).reshape(4, 128).T)
    fsbh = np.ascontiguousarray((fsb * 0.5).reshape(4, 128).T)

    qi = np.arange(LQ)
    ki = np.arange(LKV)
    in_maps = []
    for c in range(NC):
        bsel = [slots[0][c], slots[1][c]]
        wms, ams = [], []
        for b in bsel:
            valid_q = qi < al[b]
            valid_k = ki < tl[b]
            wms.append((valid_k[:, None] & valid_q[None, :]).astype(np.int32))
            ams.append(np.broadcast_to((~valid_q)[None, :], (LKV, LQ)).astype(np.float32))
        in_maps.append({
            "audioT": np.ascontiguousarray(audioT[bsel]),
            "text": np.ascontiguousarray(text[bsel]),
            "textT": np.ascontiguousarray(textT[bsel]),
            "wmask": np.stack(wms), "amask": np.stack(ams),
            "h2w": h2w, "fuwT": fuwT, "fswT": fswT,
            "vke": vke, "bm": bm, "fubh": fubh, "fsbh": fsbh,
        })

    res = bass_utils.run_bass_kernel_spmd(nc, in_maps, core_ids=list(range(NC)))
    global LAST_RESULT
    LAST_RESULT = res
    import os as _os
    if _os.environ.get("KERNEL_TIME_RUNS"):
        import time as _t
        ts = []
        for _ in range(3):
            t0 = _t.perf_counter()
            bass_utils.run_bass_kernel_spmd(nc, in_maps, core_ids=list(range(NC)))
            ts.append(_t.perf_counter() - t0)
        print(f"HW exec time: {int(min(ts)*1e9)} ns  (best-of-3 full dispatch incl host transfer)")
    out = np.zeros((B, LQ), np.float32)
    for c in range(NC):
        for s in range(2):
            out[slots[s][c]] = res.results[c]["out"][s]
    return out


if __name__ == "__main__":
    d = np.load("/root/problem/inputs.npz")
    o = kernel(**{k: d[k] for k in d.files})
    print("out", o.shape, o.dtype, float(o.min()), float(o.max()))
